# revision 13
# baseline (speedup 1.0000x reference)
"""Trainium2 Bass kernel for ConditionGuidedCrossAttention.

Sharding: 8 cores = 4 "real" heads x 2 batch-halves. Core c handles
real head h=c%4 plus positional head h+4 for batches [2*(c//4), 2*(c//4)+1].

Heads 0-3 ("real") draw Q/K purely from the conv channels; heads 4-7
("pos") draw Q/K purely from the (batch-independent) positional
projection, with the two key halves identical -> softmax([S,S]) =
0.5*[softmax(S), softmax(S)], so the pos-head attention runs on 1024
keys with a doubled denominator.

Each core computes a rank-128 partial of the output projection; the
host sums partials, adds proj bias and the residual.
"""

import numpy as np
import ml_dtypes

import concourse.bass as bass
import concourse.tile as tile
from concourse import bacc
from concourse import mybir
from concourse import bass_utils

BF16 = mybir.dt.bfloat16
F32 = mybir.dt.float32
AF = mybir.ActivationFunctionType
ALU = mybir.AluOpType

C = 512          # channels
S = 1024         # spatial positions per image (32*32)
NB = 2           # batches per core
NT = NB * S      # columns of the c-major activations
CT = 4           # 128-channel tiles
GROUPS = 32
GSIZE = 16       # channels per group
EPS = 1e-6
SC2 = float(1.0 / np.sqrt(128.0))   # scale^2 applied to logits


# ---------------------------------------------------------------- program ---

DEBUG = False


def _emit(nc, tc, t):
    """Emit the whole per-core program. `t` = dict of DRAM APs."""
    from contextlib import ExitStack
    ctx = ExitStack()
    with ctx:
        const = ctx.enter_context(tc.tile_pool(name="const", bufs=1))
        data = ctx.enter_context(tc.tile_pool(name="data", bufs=1))
        small = ctx.enter_context(tc.tile_pool(name="small", bufs=4))
        ppool = ctx.enter_context(tc.tile_pool(name="ppool", bufs=3))
        zpool = ctx.enter_context(tc.tile_pool(name="zpool", bufs=2))
        opool = ctx.enter_context(tc.tile_pool(name="opool", bufs=3))
        psum = ctx.enter_context(tc.tile_pool(name="psum", bufs=2, space="PSUM"))

        # ---- constant loads -------------------------------------------------
        def load_const(name, shape, dtype, rearr=None):
            tl = const.tile(list(shape), dtype, name=name + "_sb", tag=name)
            src = t[name]
            if rearr is not None:
                src = src.rearrange(rearr, p=128)
            nc.sync.dma_start(out=tl, in_=src)
            return tl

        wq = load_const("wq_t", [128, CT, 128], BF16, "(t p) m -> p t m")
        wkx = load_const("wkx_t", [128, CT, 128], BF16, "(t p) m -> p t m")
        wkc = load_const("wkc_t", [128, CT, 128], BF16, "(t p) m -> p t m")
        wv = load_const("wv_t", [128, CT, 128], BF16, "(t p) m -> p t m")
        wvc = load_const("wvc_t", [128, CT, 128], BF16, "(t p) m -> p t m")
        wpos = load_const("wpos_t", [128, CT, 128], BF16, "(t p) m -> p t m")
        wproj = load_const("wproj_t", [128, C], BF16)
        pos_in = load_const("pos_cm", [128, CT, S], BF16, "(t p) n -> p t n")
        gsum = load_const("gsum", [128, CT, GROUPS], F32)
        gbck = load_const("gbck", [GROUPS, CT, 128], F32)
        gam_x = load_const("gam_x", [128, CT], F32)
        bet_x = load_const("bet_x", [128, CT], F32)
        gam_c = load_const("gam_c", [128, CT], F32)
        bet_c = load_const("bet_c", [128, CT], F32)
        bq = load_const("bq", [128, 1], F32)
        bkx = load_const("bkx", [128, 1], F32)
        bkc = load_const("bkc", [128, 1], F32)
        bv = load_const("bv", [128, 1], F32)
        bvc = load_const("bvc", [128, 1], F32)
        bpos = load_const("bpos", [128, 1], F32)

        eps_t = const.tile([GROUPS, 1], F32, tag="eps")
        nc.vector.memset(eps_t, EPS)
        ones_sb = const.tile([128, 64], F32, tag="ones")
        nc.vector.memset(ones_sb, 1.0)

        x_sb = data.tile([128, CT, NT], BF16, tag="x_sb")
        nc.sync.dma_start(out=x_sb, in_=t["x_cm"].rearrange("(t p) n -> p t n", p=128))
        c_sb = data.tile([128, CT, NT], BF16, tag="c_sb")
        nc.sync.dma_start(out=c_sb, in_=t["cond_cm"].rearrange("(t p) n -> p t n", p=128))

        # ---- group-norm stats + apply --------------------------------------
        def norm(src_sb, gam, bet, dst_name):
            """GroupNorm src_sb -> new [128, CT, NT] bf16 tile."""
            # per-(channel,batch) stats
            statassy = small.tile([128, CT, NB, 3], F32, name=dst_name + "_stat", tag=dst_name + "_stat")
            for ct in range(CT):
                for b in range(NB):
                    bn = small.tile([128, 2, 6], F32, name=dst_name + "_bn", tag="bn")
                    for j in range(2):
                        nc.vector.bn_stats(
                            out=bn[:, j, :],
                            in_=src_sb[:, ct, b * S + j * 512: b * S + (j + 1) * 512],
                        )
                    nc.vector.bn_aggr(out=statassy[:, ct, b, 0:2], in_=bn)
                    nc.vector.tensor_mul(
                        statassy[:, ct, b, 2:3],
                        statassy[:, ct, b, 0:1],
                        statassy[:, ct, b, 0:1],
                    )
            # combine 16 channels -> group stats (PE contraction)
            gstats = psum.tile([GROUPS, NB, 3], F32, name=dst_name + "_gst", tag="A")
            for ct in range(CT):
                nc.tensor.matmul(
                    gstats, gsum[:, ct, :], statassy[:, ct, :, :],
                    start=(ct == 0), stop=(ct == CT - 1),
                )
            gst = small.tile([GROUPS, NB, 3], F32, name=dst_name + "_gsb", tag=dst_name + "_gsb")
            nc.vector.tensor_copy(gst, gstats)
            # var_g = E[var] + E[m^2] - mean_g^2
            gvar = small.tile([GROUPS, NB], F32, name=dst_name + "_gvar", tag=dst_name + "_gvar")
            nc.vector.tensor_add(gvar, gst[:, :, 1], gst[:, :, 2])
            gm2 = small.tile([GROUPS, NB], F32, name=dst_name + "_gm2", tag=dst_name + "_gm2")
            nc.vector.tensor_mul(gm2, gst[:, :, 0], gst[:, :, 0])
            gvar2 = small.tile([GROUPS, NB], F32, name=dst_name + "_gvar2", tag=dst_name + "_gvar2")
            nc.vector.tensor_sub(gvar2, gvar, gm2)
            gstd = small.tile([GROUPS, NB], F32, name=dst_name + "_gstd", tag=dst_name + "_gstd")
            nc.scalar.activation(gstd, gvar2, AF.Sqrt, bias=eps_t, scale=1.0)
            grstd = small.tile([GROUPS, NB], F32, name=dst_name + "_grstd", tag=dst_name + "_grstd")
            nc.vector.reciprocal(grstd, gstd)
            # broadcast rhs [32, NB, 2] = (mean, rstd)
            brhs = small.tile([GROUPS, NB, 2], F32, name=dst_name + "_brhs", tag=dst_name + "_brhs")
            nc.vector.tensor_copy(brhs[:, :, 0], gst[:, :, 0])
            nc.vector.tensor_copy(brhs[:, :, 1], grstd)
            # scale/shift per channel, then apply
            dst = data.tile([128, CT, NT], BF16, name=dst_name, tag=dst_name)
            for ct in range(CT):
                bc = psum.tile([128, NB, 2], F32, name=dst_name + "_bc", tag="A")
                nc.tensor.matmul(bc, gbck[:, ct, :], brhs, start=True, stop=True)
                for b in range(NB):
                    sc = small.tile([128, 1], F32, name=dst_name + "_sc", tag="sc")
                    nc.vector.tensor_mul(sc, bc[:, b, 1:2], gam[:, ct:ct + 1])
                    tm = small.tile([128, 1], F32, name=dst_name + "_tm", tag="tm")
                    nc.vector.tensor_mul(tm, bc[:, b, 0:1], sc)
                    sh = small.tile([128, 1], F32, name=dst_name + "_sh", tag="sh")
                    nc.vector.tensor_sub(sh, bet[:, ct:ct + 1], tm)
                    nc.vector.tensor_scalar(
                        out=dst[:, ct, b * S:(b + 1) * S],
                        in0=src_sb[:, ct, b * S:(b + 1) * S],
                        scalar1=sc, scalar2=sh,
                        op0=ALU.mult, op1=ALU.add,
                    )
            return dst

        xn = norm(x_sb, gam_x, bet_x, "xn")
        cn = norm(c_sb, gam_c, bet_c, "cn")
        if DEBUG:
            nc.sync.dma_start(out=t["xn_dbg"].rearrange("(t p) n -> p t n", p=128), in_=xn)

        # ---- 1x1 convs ------------------------------------------------------
        def conv(w_sb, rhs_sb, bias, ncols, out_name):
            out_sb = data.tile([128, ncols], BF16, name=out_name, tag=out_name)
            for nh in range(ncols // S):
                ps = psum.tile([128, S], F32, name=out_name + "_ps", tag="A")
                for ct in range(CT):
                    for half in range(2):
                        nc.tensor.matmul(
                            ps[:, half * 512:(half + 1) * 512],
                            w_sb[:, ct, :],
                            rhs_sb[:, ct, nh * S + half * 512: nh * S + (half + 1) * 512],
                            start=(ct == 0), stop=(ct == CT - 1),
                        )
                nc.vector.tensor_scalar_add(
                    out_sb[:, nh * S:(nh + 1) * S], ps, bias)
            return out_sb

        q_sb = conv(wq, xn, bq, NT, "q_sb")
        if DEBUG:
            nc.sync.dma_start(out=t["q_dbg"], in_=q_sb)
        kx_sb = conv(wkx, xn, bkx, NT, "kx_sb")
        kc_sb = conv(wkc, cn, bkc, NT, "kc_sb")
        v_sb = conv(wv, xn, bv, NT, "v_sb")
        vc_sb = conv(wvc, cn, bvc, NT, "vc_sb")
        posp = conv(wpos, pos_in, bpos, S, "posp")
        if DEBUG:
            nc.sync.dma_start(out=t["posp_dbg"], in_=posp)
            nc.sync.dma_start(out=t["kx_dbg"], in_=kx_sb)
            nc.sync.dma_start(out=t["v_dbg"], in_=v_sb)

        # ---- V assembly: [k, dv] layout via DMA transpose -------------------
        vsum = data.tile([128, NT], BF16, name="vsum", tag="vsum")
        nc.vector.tensor_add(vsum[64:128, :], v_sb[64:128, :], vc_sb[64:128, :])

        vT_real = data.tile([128, NB, 16, 80], BF16, tag="vT_real")
        vT_pos = data.tile([128, NB, 8, 128], BF16, tag="vT_pos")
        nc.vector.memset(vT_real[:, :, :, 64:65], 1.0)
        nc.vector.memset(vT_pos[:, :, :, 0:32], 0.0)
        nc.vector.memset(vT_pos[:, :, :, 32:33], 2.0)
        nc.vector.memset(vT_pos[:, :, :, 33:64], 0.0)
        for b in range(NB):
            for kt in range(16):
                src = v_sb if kt < 8 else vc_sb
                nc.sync.dma_start_transpose(
                    out=vT_real[:, b, kt, 0:64],
                    in_=src[0:64, b * S + (kt % 8) * 128: b * S + (kt % 8) * 128 + 128],
                )
            for kt in range(8):
                nc.sync.dma_start_transpose(
                    out=vT_pos[:, b, kt, 64:128],
                    in_=vsum[64:128, b * S + kt * 128: b * S + kt * 128 + 128],
                )

        # ---- attention ------------------------------------------------------
        att_sb = data.tile([128, NT], BF16, name="att_sb", tag="att_sb")

        def finish_real(av, b):
            """av: [65, S] PSUM (row 64 = Z). att[0:64] = av[0:64]/Z."""
            zr = zpool.tile([65, S], F32, name="zr", tag="zr")
            nc.vector.reciprocal(out=zr[64:65, :], in_=av[64:65, :])
            zb_ps = psum.tile([64, S], F32, name="zb_ps", tag="A")
            for hf in range(2):
                nc.tensor.matmul(
                    zb_ps[:, hf * 512:(hf + 1) * 512],
                    ones_sb[64:65, :], zr[64:65, hf * 512:(hf + 1) * 512],
                    start=True, stop=True)
            zb = zpool.tile([64, S], F32, name="zb", tag="zb")
            nc.vector.tensor_copy(zb, zb_ps)
            nc.vector.tensor_mul(
                att_sb[0:64, b * S:(b + 1) * S], av[0:64, :], zb)
            return zb

        def finish_pos(av, b):
            """av: [128, S] PSUM (row 63 = Z, rows 64:128 = out)."""
            zr = zpool.tile([65, S], F32, name="zr2", tag="zr")
            nc.vector.reciprocal(out=zr[32:33, :], in_=av[32:33, :])
            zb_ps = psum.tile([128, S], F32, name="zb2_ps", tag="A")
            for hf in range(2):
                nc.tensor.matmul(
                    zb_ps[64:128, hf * 512:(hf + 1) * 512],
                    ones_sb[32:33, :], zr[32:33, hf * 512:(hf + 1) * 512],
                    start=True, stop=True)
            zb = zpool.tile([128, S], F32, name="zb2", tag="zb2")
            nc.vector.tensor_copy(zb[64:128, :], zb_ps[64:128, :])
            nc.vector.tensor_mul(
                att_sb[64:128, b * S:(b + 1) * S], av[64:128, :], zb[64:128, :])
            return zb


        def dbg_fin(av, zb, zrow, r0, r1):
            di = dbg_fin.dbg_i
            dbg_fin.dbg_i += 1
            avc = opool.tile([128, S], F32, name="avc", tag="avc")
            nc.vector.tensor_copy(avc[r0:r1, :], av[r0:r1, :])
            nc.vector.tensor_copy(avc[zrow:zrow + 1, :], av[zrow:zrow + 1, :])
            nc.sync.dma_start(out=t["av_dbg"].rearrange("(i p) n -> i p n", p=128)[di, r0:r1], in_=avc[r0:r1, :])
            nc.sync.dma_start(out=t["av_dbg"].rearrange("(i p) n -> i p n", p=128)[di, zrow:zrow + 1], in_=avc[zrow:zrow + 1, :])
            nc.sync.dma_start(out=t["zb_dbg"].rearrange("(i p) n -> i p n", p=64)[di], in_=zb[r0:r1, :] if r0 else zb)
        dbg_fin.dbg_i = 0

        # real head: 2048 keys
        for b in range(NB):
            av = psum.tile([65, S], F32, name="av_r", tag="av")
            for kt in range(16):
                ksrc = kx_sb if kt < 8 else kc_sb
                kblk = ksrc[:, b * S + (kt % 8) * 128: b * S + (kt % 8) * 128 + 128]
                sT = psum.tile([128, S], F32, name="sT", tag="A")
                for half in range(2):
                    nc.tensor.matmul(
                        sT[:, half * 512:(half + 1) * 512],
                        kblk, q_sb[:, b * S + half * 512: b * S + (half + 1) * 512],
                        start=True, stop=True,
                    )
                pT = ppool.tile([128, S], BF16, name="pT", tag="pT")
                nc.scalar.activation(pT, sT, AF.Exp, scale=SC2)
                for half in range(2):
                    nc.tensor.matmul(
                        av[:, half * 512:(half + 1) * 512],
                        vT_real[:, b, kt, 0:65],
                        pT[:, half * 512:(half + 1) * 512],
                        start=(kt == 0), stop=(kt == 15),
                    )
            fin_zb = finish_real(av, b)
            if DEBUG:
                dbg_fin(av, fin_zb, 64, 0, 64)

        # pos head: 1024 keys shared across batches, doubled denominator
        avp = [psum.tile([128, S], F32, name=f"av_p{b}", tag="av") for b in range(NB)]
        for kt in range(8):
            kblk = posp[:, kt * 128: kt * 128 + 128]
            sT = psum.tile([128, S], F32, name="sTp", tag="A")
            for half in range(2):
                nc.tensor.matmul(
                    sT[:, half * 512:(half + 1) * 512],
                    kblk, posp[:, half * 512:(half + 1) * 512],
                    start=True, stop=True,
                )
            pT = ppool.tile([128, S], BF16, name="pTp", tag="pT")
            nc.scalar.activation(pT, sT, AF.Exp, scale=SC2)
            for b in range(NB):
                for half in range(2):
                    nc.tensor.matmul(
                        avp[b][:, half * 512:(half + 1) * 512],
                        vT_pos[:, b, kt, :],
                        pT[:, half * 512:(half + 1) * 512],
                        start=(kt == 0), stop=(kt == 7),
                    )
        for b in range(NB):
            fin_zb = finish_pos(avp[b], b)
            if DEBUG:
                dbg_fin(avp[b], fin_zb, 32, 64, 128)

        if DEBUG:
            nc.sync.dma_start(out=t["att_dbg"], in_=att_sb)
            nc.sync.dma_start(out=t["vt_dbg"].rearrange("p (b k c) -> p b k c", b=NB, k=16), in_=vT_real)

        # ---- output projection (partial: this core's 128 channels) ---------
        out_r = t["out_part"].rearrange("(mt p) n -> p mt n", p=128)
        for b in range(NB):
            for mt in range(4):
                ps = psum.tile([128, S], F32, name="proj_ps", tag="A")
                for half in range(2):
                    nc.tensor.matmul(
                        ps[:, half * 512:(half + 1) * 512],
                        wproj[:, mt * 128:(mt + 1) * 128],
                        att_sb[:, b * S + half * 512: b * S + (half + 1) * 512],
                        start=True, stop=True,
                    )
                o_sb = opool.tile([128, S], F32, name="o_sb", tag="o")
                nc.scalar.copy(o_sb, ps)
                nc.sync.dma_start(out=out_r[:, mt, b * S:(b + 1) * S], in_=o_sb)


def build_program():
    nc = bacc.Bacc("TRN2", target_bir_lowering=False, debug=False, num_devices=8)
    t = {}

    def inp(name, shape, dtype):
        t[name] = nc.dram_tensor(name, list(shape), dtype, kind="ExternalInput").ap()

    inp("x_cm", (C, NT), BF16)
    inp("cond_cm", (C, NT), BF16)
    for w in ("wq_t", "wkx_t", "wkc_t", "wv_t", "wvc_t", "wpos_t"):
        inp(w, (C, 128), BF16)
    inp("wproj_t", (128, C), BF16)
    inp("pos_cm", (C, S), BF16)
    inp("gsum", (128, CT, GROUPS), F32)
    inp("gbck", (GROUPS, CT, 128), F32)
    for g in ("gam_x", "bet_x", "gam_c", "bet_c"):
        inp(g, (128, CT), F32)
    for bname in ("bq", "bkx", "bkc", "bv", "bvc", "bpos"):
        inp(bname, (128, 1), F32)
    t["out_part"] = nc.dram_tensor(
        "out_part", [C, NT], F32, kind="ExternalOutput").ap()
    if DEBUG:
        def outp(name, shape, dtype):
            t[name] = nc.dram_tensor(name, list(shape), dtype, kind="ExternalOutput").ap()
        outp("xn_dbg", (C, NT), BF16)
        outp("q_dbg", (128, NT), BF16)
        outp("kx_dbg", (128, NT), BF16)
        outp("v_dbg", (128, NT), BF16)
        outp("posp_dbg", (128, S), BF16)
        outp("att_dbg", (128, NT), BF16)
        outp("vt_dbg", (128, NB * 16 * 80), BF16)
        outp("av_dbg", (4 * 128, S), F32)
        outp("zb_dbg", (4 * 64, S), F32)

    with tile.TileContext(nc) as tc:
        _emit(nc, tc, t)
    nc.compile()
    return nc


# ------------------------------------------------------------------- host ---

def _bf16(a):
    return np.ascontiguousarray(np.asarray(a, np.float32)).astype(ml_dtypes.bfloat16)


def _f32(a):
    return np.ascontiguousarray(np.asarray(a, np.float32))


def host_prepare(inputs):
    x = np.asarray(inputs["x"], np.float32)        # (4, 512, 32, 32)
    cond = np.asarray(inputs["condition"], np.float32)
    B = x.shape[0]
    xf = x.reshape(B, C, S)
    cf = cond.reshape(B, C, S)

    qkv_w = np.asarray(inputs["qkv_w"], np.float32)
    ckv_w = np.asarray(inputs["ckv_w"], np.float32)
    pos_w = np.asarray(inputs["pos_w"], np.float32)
    proj_w = np.asarray(inputs["proj_w"], np.float32)
    pos_emb = np.asarray(inputs["pos_emb"], np.float32)[:, :S]
    qkv_b = np.asarray(inputs["qkv_b"], np.float32)
    ckv_b = np.asarray(inputs["ckv_b"], np.float32)
    pos_b = np.asarray(inputs["pos_b"], np.float32)

    # group matrices (same for every core)
    ch = np.arange(128)
    gsum = np.zeros((128, CT, GROUPS), np.float32)
    gbck = np.zeros((GROUPS, CT, 128), np.float32)
    for ct in range(CT):
        g_of_p = 8 * ct + ch // GSIZE
        gsum[ch, ct, g_of_p] = 1.0 / GSIZE
        gbck[g_of_p, ct, ch] = 1.0
    gam = {}
    for nm, key in (("gam_x", "norm_g"), ("bet_x", "norm_b"),
                    ("gam_c", "cnorm_g"), ("bet_c", "cnorm_b")):
        gam[nm] = _f32(np.asarray(inputs[key], np.float32).reshape(CT, 128).T)

    in_maps = []
    for core in range(8):
        h = core % 4
        bh = core // 4
        bs = [2 * bh, 2 * bh + 1]
        vrows = list(range(1024 + 64 * h, 1024 + 64 * h + 64)) + \
                list(range(1024 + 64 * (h + 4), 1024 + 64 * (h + 4) + 64))
        vcrows = list(range(512 + 64 * h, 512 + 64 * h + 64)) + \
                 list(range(512 + 64 * (h + 4), 512 + 64 * (h + 4) + 64))
        pcols = list(range(64 * h, 64 * h + 64)) + \
                list(range(64 * (h + 4), 64 * (h + 4) + 64))
        m = {
            "x_cm": _bf16(np.concatenate([xf[bs[0]], xf[bs[1]]], 1)),
            "cond_cm": _bf16(np.concatenate([cf[bs[0]], cf[bs[1]]], 1)),
            "wq_t": _bf16(qkv_w[h * 128:(h + 1) * 128].T),
            "wkx_t": _bf16(qkv_w[512 + h * 128: 512 + (h + 1) * 128].T),
            "wkc_t": _bf16(ckv_w[h * 128:(h + 1) * 128].T),
            "wv_t": _bf16(qkv_w[vrows].T),
            "wvc_t": _bf16(ckv_w[vcrows].T),
            "wpos_t": _bf16(pos_w[h * 128:(h + 1) * 128].T),
            "wproj_t": _bf16(proj_w[:, pcols].T),
            "pos_cm": _bf16(pos_emb),
            "gsum": gsum, "gbck": gbck,
            "gam_x": gam["gam_x"], "bet_x": gam["bet_x"],
            "gam_c": gam["gam_c"], "bet_c": gam["bet_c"],
            "bq": _f32(qkv_b[h * 128:(h + 1) * 128]).reshape(128, 1),
            "bkx": _f32(qkv_b[512 + h * 128: 512 + (h + 1) * 128]).reshape(128, 1),
            "bkc": _f32(ckv_b[h * 128:(h + 1) * 128]).reshape(128, 1),
            "bv": _f32(qkv_b[vrows]).reshape(128, 1),
            "bvc": _f32(ckv_b[vcrows]).reshape(128, 1),
            "bpos": _f32(pos_b[h * 128:(h + 1) * 128]).reshape(128, 1),
        }
        in_maps.append(m)
    return in_maps


def gather(results, inputs):
    x = np.asarray(inputs["x"], np.float32)
    proj_b = np.asarray(inputs["proj_b"], np.float32)
    B = x.shape[0]
    out = x.reshape(B, C, S).copy()
    out += proj_b[None, :, None]
    for core in range(8):
        bh = core // 4
        part = np.asarray(results[core]["out_part"], np.float32)  # (512, 2048)
        for j in range(NB):
            out[2 * bh + j] += part[:, j * S:(j + 1) * S]
    return out.reshape(B, C, 32, 32).astype(np.float32)


_PROGRAM = None


def kernel(**inputs):
    global _PROGRAM
    if _PROGRAM is None:
        _PROGRAM = build_program()
    in_maps = host_prepare(inputs)
    res = bass_utils.run_bass_kernel_spmd(_PROGRAM, in_maps, core_ids=list(range(8)))
    return gather(res.results, inputs)
